# revision 8
# baseline (speedup 1.0000x reference)
"""CrossEntropyLossWithProb on 8 trn2 NeuronCores.

loss = -mean(log(max(probs[i, labels[i]], 1e-8)))  over i in [0, 8192)

Row-sharded across 8 cores; each core gathers its 1024 addressed
probabilities (4 KB of the 128 MB shard) via 8 indirect DMAs -- the HW
indirect-DMA contract is one dynamic offset per offset-AP partition
(verified against the BIR simulator; multi-column offset APs silently
degrade to consecutive-run fetches), so 1024 arbitrary offsets need
8 x 128-offset instructions. Their SWDGE preps serialize on the Pool
engine (994 ns fixed each) and dominate the kernel.

Optimizations vs the naive pipeline:
  - idx load DMA is hoisted in front of the framework's const-init
    barrier (it depends on nothing), so its ~2.3 us latency overlaps
    the preamble and the first gather prep starts ~600 ns earlier.
  - clamp is folded into the gathers: g_t is pre-filled with 1e-8 and
    the gathers CCE-add into it, giving probs[idx] + 1e-8 in-flight
    (within 1e-8 of max(probs[idx], 1e-8); rel-err impact ~1e-8).
    This removes the DVE clamp stage entirely.
  - ln runs in two waves: cols 0-6 with accumulate hide under the
    later gather preps; col 7 is a single-element ln written straight
    into its accumulator slot (no 187 ns accumulator-read).
  - host sums the [128, 2] partials (replaces all-reduce).

  SP  : dma idx[128,8] (pre-barrier), later dma acc[128,2] -> out
  PL  : memset g_t=1e-8; 8 gathers (CCE add) -> s_g; tail dma_reset +
        sem_clear after s_out (race-free: every semaphore's last
        consumer has retired by then)
  ACT : ln waves -> acc
"""

import numpy as np

import concourse.bacc as bacc
import concourse.bass as bass
import concourse.mybir as mybir
from concourse.bass import compact_to_ranges

B, V = 8192, 32000
N_CORES = 8
BS = B // N_CORES
P, C = 128, BS // 128
CLIP = 1e-8

_cached_nc = None


def build_nc(detect_races=False):
    global _cached_nc
    if _cached_nc is not None and not detect_races:
        return _cached_nc

    nc = bacc.Bacc("TRN2", target_bir_lowering=False, debug=False,
                   num_devices=N_CORES,
                   detect_race_conditions=detect_races)
    probs = nc.dram_tensor("probs", [BS, V], mybir.dt.float32,
                           kind="ExternalInput")
    idx = nc.dram_tensor("idx", [P, C], mybir.dt.int32, kind="ExternalInput")
    out = nc.dram_tensor("out", [P, 2], mybir.dt.float32,
                         kind="ExternalOutput")

    probs_flat = bass.AP(probs, 0, [[1, BS * V], [1, 1]])

    with (
        nc.sbuf_tensor("idx_t", [P, C], mybir.dt.int32) as idx_t,
        nc.sbuf_tensor("g_t", [P, C], mybir.dt.float32) as g_t,
        nc.sbuf_tensor("ll_t", [P, C], mybir.dt.float32) as ll_t,
        nc.sbuf_tensor("acc_t", [P, 2], mybir.dt.float32) as acc_t,
        nc.semaphore("s_idx") as s_idx,
        nc.semaphore("s_g") as s_g,
        nc.semaphore("s_act") as s_act,
        nc.semaphore("s_out") as s_out,
    ):
        # SP stream: idx load (hoisted pre-barrier below), then the
        # output store once both ln waves have landed.
        idx_dma = nc.sync.dma_start(idx_t[:], idx.ap()).then_inc(s_idx, 16)
        nc.sync.wait_ge(s_act, 2)
        # No SP wait on s_out: PL's tail wait covers output landing, and a
        # second waiter could still be polling when PL clears the sem.
        nc.sync.dma_start(out.ap(), acc_t[:]).then_inc(s_out, 16)

        # PL stream: prefill g_t with the clamp floor, then 8 gathers
        # (one offset per partition each) CCE-adding into it. Program
        # order on Pool makes the memset safely precede the DMAs.
        nc.gpsimd.memset(g_t[:], CLIP)
        nc.gpsimd.wait_ge(s_idx, 16)
        for c in range(C):
            nc.gpsimd.indirect_dma_start(
                out=g_t[:, c:c + 1], out_offset=None, in_=probs_flat,
                in_offset=bass.IndirectOffsetOnAxis(
                    ap=idx_t[:, c:c + 1], axis=0),
                compute_op=mybir.AluOpType.add,
            ).then_inc(s_g, 16)

        # ACT stream: ln cols 0..6 (hidden under the later gather preps)
        # with accumulate; then the last column alone, written straight
        # into its accumulator slot -- a 1-wide row sum is the identity,
        # so no accum_out (saves the accumulator-read latency).
        nc.scalar.wait_ge(s_g, 16 * (C - 1))
        nc.scalar.activation(ll_t[:, :C - 1], g_t[:, :C - 1],
                             mybir.ActivationFunctionType.Ln,
                             accum_out=acc_t[:, 0:1]).then_inc(s_act, 1)
        nc.scalar.wait_ge(s_g, 16 * C)
        nc.scalar.activation(acc_t[:, 1:2], g_t[:, C - 1:C],
                             mybir.ActivationFunctionType.Ln)\
            .then_inc(s_act, 1)

        # PL tail: by s_out>=16 every other engine's final sem value has
        # been reached and consumed, so resetting here is race-free.
        nc.gpsimd.wait_ge(s_out, 16)
        sem_ids = sorted(s.num for s in (s_idx, s_g, s_act, s_out))
        for sem_range in compact_to_ranges(sem_ids):
            nc.gpsimd.dma_reset(sem_range)
            nc.gpsimd.sem_clear(sem_range)

    # Hoist the idx load ahead of the framework's const-init barrier in
    # the SP queue: it has no dependencies, so its DMA latency overlaps
    # the startup barrier instead of queueing behind it.
    blk = nc.m.functions[0].blocks[0].instructions
    sp = mybir.EngineType.SP
    target = idx_dma.ins if hasattr(idx_dma, "ins") else idx_dma
    ii = next(i for i, x in enumerate(blk)
              if isinstance(x, mybir.InstDMACopy) and x.engine == sp)
    di = next(i for i, x in enumerate(blk)
              if isinstance(x, mybir.InstDrain) and x.engine == sp)
    if di < ii:
        inst = blk.pop(ii)
        blk.insert(di, inst)

    nc.compile()
    if not detect_races:
        _cached_nc = nc
    return nc


def make_in_maps(probs, labels):
    probs = np.ascontiguousarray(np.asarray(probs), dtype=np.float32)
    labels = np.asarray(labels).astype(np.int64, copy=False)
    assert probs.shape == (B, V) and labels.shape == (B,)
    row = np.arange(BS, dtype=np.int64) * V
    in_maps = []
    for c in range(N_CORES):
        lb = labels[c * BS:(c + 1) * BS]
        flat = (row + lb).astype(np.int32).reshape(P, C)
        in_maps.append({"probs": probs[c * BS:(c + 1) * BS], "idx": flat})
    return in_maps


def kernel(probs, labels):
    from concourse.bass_utils import run_bass_kernel_spmd
    nc = build_nc()
    in_maps = make_in_maps(probs, labels)
    res = run_bass_kernel_spmd(nc, in_maps, core_ids=list(range(N_CORES)))
    total = np.float64(0.0)
    for r in res.results:
        total += np.float64(r["out"].sum(dtype=np.float64))
    return np.array(-total / B, dtype=np.float32)
